# revision 1
# baseline (speedup 1.0000x reference)
"""CoreAttention on 8 Trainium2 cores.

Sharding: 32 (batch, head) pairs -> 4 heads per core (cores 0-3: batch 0,
cores 4-7: batch 1). Per core, per head: scores^T = K Q^T in [t, s]
orientation with bf16 operands (fp32 PSUM accumulate), exp on ACT writing
bf16, mask applied as a bf16 multiply on DVE (2x perf mode), P@V and
column-sums as bf16 matmuls accumulating in PSUM, normalization via
reciprocal_approx_fast + gpsimd partition_broadcast + DVE multiply.
Software-pipelined: scores/exp/mask for block i+1 are issued before the
PV/sums/normalize tail of block i so the PE never stalls on the DVE chain.
Host side only slices/transposes/casts inputs (layout prep).
"""
import sys, math
import numpy as np

sys.path.insert(0, "/opt/trn_rl_repo")

SQ, B, NH, HN = 2048, 2, 16, 128
NCORES = 8
HPC = 4                      # heads per core
TC = SQ // 128               # 16 t-chunks
SBLK = 512                   # s-block width
NSB = SQ // SBLK             # 4 s-blocks
SCALE = 1.0 / math.sqrt(128.0)   # COEFF / NORM_FACTOR = 1/sqrt(hn)
KDVE = 6                         # t-chunks whose column-sums go via DVE adds
DEPTH = 2                        # software pipeline depth (F blocks ahead of B)

_CACHE = {}


def _build(repeat=1):
    import concourse.bacc as bacc
    import concourse.tile as tile
    from concourse import mybir

    F32, BF16 = mybir.dt.float32, mybir.dt.bfloat16
    EXP = mybir.ActivationFunctionType.Exp

    nc = bacc.Bacc(None, target_bir_lowering=False)
    qT_d = nc.dram_tensor("qT", [HPC, HN, SQ], BF16, kind="ExternalInput")
    kT_d = nc.dram_tensor("kT", [HPC, HN, SQ], BF16, kind="ExternalInput")
    v_d = nc.dram_tensor("v", [HPC, SQ, HN], BF16, kind="ExternalInput")
    keep_d = nc.dram_tensor("keepT", [SQ, SQ], BF16, kind="ExternalInput")
    ctxT_d = nc.dram_tensor("ctxT", [HPC, HN, SQ], BF16, kind="ExternalOutput")

    with tile.TileContext(nc) as tc:
        with (
            tc.tile_pool(name="sbkeep", bufs=1) as sbkeep,
            tc.tile_pool(name="const", bufs=1) as const,
            tc.tile_pool(name="sbqkv", bufs=2) as sbqkv,
            tc.tile_pool(name="sbpt", bufs=DEPTH + 1) as sbpt,
            tc.tile_pool(name="sbacc", bufs=DEPTH + 1) as sbacc,
            tc.tile_pool(name="sbe", bufs=3) as sbe,
            tc.tile_pool(name="sbmisc", bufs=2) as sbmisc,
            tc.tile_pool(name="pst", bufs=2, space="PSUM") as pst,
            tc.tile_pool(name="psc", bufs=2, space="PSUM") as psc,
            tc.tile_pool(name="pss", bufs=1, space="PSUM") as pss,
            tc.tile_pool(name="pwm", bufs=1, space="PSUM") as pwm,
        ):
            keep_t = sbkeep.tile([128, TC, SQ], BF16, tag="keep")
            keep_r = keep_d.rearrange("(c p) s -> p c s", p=128)

            ones_b = const.tile([128, 1], BF16, tag="ob")
            nc.vector.memset(ones_b[:], 1.0)
            warm_src = const.tile([128, SBLK], BF16, tag="warm")
            nc.vector.memset(warm_src[:], 0.0)
            warm_e = const.tile([128, 16], BF16, tag="warme")

            def emit_front(h, sb, qT_t, kT_t):
                """scores -> exp -> mask for (h, sb); returns (pt, acc).

                Chunks 0..KDVE-1 of the column-sum reduction run here as
                bf16 adds on the DVE (acc), interleaved with the mask
                multiplies; the PE later contracts only chunks KDVE..15
                plus one join matmul over acc.
                """
                s0 = sb * SBLK
                pt = sbpt.tile([128, TC, SBLK], BF16, tag="pt")
                acc = sbacc.tile([128, SBLK], BF16, tag="acc")
                for g in range(TC // 2):
                    st = pst.tile([128, 2, SBLK], F32, tag="st")
                    for j in range(2):
                        ti = 2 * g + j
                        nc.tensor.matmul(
                            st[:, j, :],
                            kT_t[:, 128 * ti:128 * (ti + 1)],
                            qT_t[:, s0:s0 + SBLK],
                            start=True, stop=True)
                    e16 = sbe.tile([128, 2, SBLK], BF16, tag="e")
                    nc.scalar.activation(e16[:], st[:], EXP, scale=SCALE)
                    nc.vector.tensor_mul(
                        pt[:, 2 * g:2 * g + 2, :], e16[:],
                        keep_t[:, 2 * g:2 * g + 2, s0:s0 + SBLK])
                    if g == 0:
                        nc.vector.tensor_add(acc[:], pt[:, 0, :], pt[:, 1, :])
                    else:
                        for j in range(2):
                            ti = 2 * g + j
                            if ti < KDVE:
                                nc.vector.tensor_add(acc[:], acc[:],
                                                     pt[:, ti, :])
                return pt, acc

            def emit_back(h, sb, pt, acc, v_t, nsplit=1):
                """sums -> PV -> normalize -> store for (h, sb)."""
                s0 = sb * SBLK
                sums_p = pss.tile([1, SBLK], F32, tag="sums")
                for ti in range(KDVE, TC):
                    nc.tensor.matmul(sums_p[:], ones_b[:], pt[:, ti, :],
                                     start=(ti == KDVE), stop=False)
                nc.tensor.matmul(sums_p[:], ones_b[:], acc[:],
                                 start=False, stop=True)
                ctx_p = psc.tile([128, SBLK], F32, tag="ctx")
                for ti in range(TC):
                    nc.tensor.matmul(ctx_p[:], v_t[:, ti, :], pt[:, ti, :],
                                     start=(ti == 0), stop=(ti == TC - 1))
                w = SBLK // nsplit
                for o in range(0, SBLK, w):
                    recip = sbmisc.tile([1, w], F32, tag="recip")
                    nc.vector.reciprocal_approx_fast(recip[:],
                                                     sums_p[:, o:o + w])
                    rep_s = sbmisc.tile([128, w], F32, tag="reps")
                    nc.gpsimd.partition_broadcast(rep_s[:], recip[:])
                    ctx_s = sbmisc.tile([128, w], BF16, tag="ctxs")
                    nc.vector.tensor_mul(ctx_s[:], ctx_p[:, o:o + w], rep_s[:])
                    nc.sync.dma_start(out=ctxT_d[h, :, s0 + o:s0 + o + w],
                                      in_=ctx_s[:])

            def body(_iv=None):
                # warm the PE clock (HAM) and the ACT exp table with dummy
                # ops that only depend on the memset, while the first DMAs
                # land
                warm_p = pwm.tile([1, SBLK], F32, tag="warmp")
                for _ in range(6):
                    nc.tensor.matmul(warm_p[:], ones_b[:], warm_src[:],
                                     start=True, stop=True)
                nc.scalar.activation(warm_e[:], warm_src[:, 0:16], EXP,
                                     scale=SCALE)

                pendings = []   # [(h, sb, pt, acc, v_t), ...]
                qkv = {}
                for h in range(HPC):
                    qT_t = sbqkv.tile([128, SQ], BF16, tag="qT")
                    kT_t = sbqkv.tile([128, SQ], BF16, tag="kT")
                    v_t = sbqkv.tile([128, TC, HN], BF16, tag="v")
                    if h == 0:
                        # fine-grained first loads: the ti=0 score matmul
                        # needs only kT columns 0:512 and qT s-block 0
                        for c in range(NSB):
                            nc.sync.dma_start(
                                out=kT_t[:, SBLK * c:SBLK * (c + 1)],
                                in_=kT_d[h][:, SBLK * c:SBLK * (c + 1)])
                            nc.sync.dma_start(
                                out=qT_t[:, SBLK * c:SBLK * (c + 1)],
                                in_=qT_d[h][:, SBLK * c:SBLK * (c + 1)])
                        # mask columns for s-block 0 next (block (0,0)
                        # needs them first), then v0, then the rest — so
                        # demand never outruns the ~400 GB/s HBM supply
                        # during the first ~35 us
                        nc.sync.dma_start(out=keep_t[:, :, 0:SBLK],
                                          in_=keep_r[:, :, 0:SBLK])
                    else:
                        nc.sync.dma_start(out=qT_t[:], in_=qT_d[h])
                        nc.sync.dma_start(out=kT_t[:], in_=kT_d[h])
                    nc.sync.dma_start(out=v_t[:],
                                      in_=v_d[h].rearrange("(c p) d -> p c d", p=128))
                    if h == 0:
                        for sb in range(1, NSB):
                            s0 = sb * SBLK
                            nc.sync.dma_start(out=keep_t[:, :, s0:s0 + SBLK],
                                              in_=keep_r[:, :, s0:s0 + SBLK])
                    qkv[h] = (qT_t, kT_t, v_t)
                    for sb in range(NSB):
                        pt, acc = emit_front(h, sb, qT_t, kT_t)
                        pendings.append((h, sb, pt, acc, v_t))
                        if len(pendings) > DEPTH:
                            emit_back(*pendings.pop(0))
                for i, p in enumerate(pendings):
                    emit_back(*p, nsplit=2 * (i + 1))

            if repeat == 1:
                body()
            else:
                with tc.For_i(0, repeat, 1):
                    body()
    nc.compile()
    return nc


def _get_nc(repeat=1):
    if repeat not in _CACHE:
        _CACHE[repeat] = _build(repeat)
    return _CACHE[repeat]


def _make_in_maps(query_layer, key_layer, value_layer, attention_mask):
    import ml_dtypes
    bf16 = ml_dtypes.bfloat16
    q = np.asarray(query_layer, dtype=np.float32)
    k = np.asarray(key_layer, dtype=np.float32)
    v = np.asarray(value_layer, dtype=np.float32)
    m = np.asarray(attention_mask)
    in_maps = []
    for c in range(NCORES):
        b = c // 4
        h0 = 4 * (c % 4)
        hs = slice(h0, h0 + HPC)
        qT = np.ascontiguousarray(
            q[:, b, hs, :].transpose(1, 2, 0)).astype(bf16)    # [4,hn,sq]
        kT = np.ascontiguousarray(
            k[:, b, hs, :].transpose(1, 2, 0)).astype(bf16)
        vv = np.ascontiguousarray(
            v[:, b, hs, :].transpose(1, 0, 2)).astype(bf16)    # [4,sq,hn]
        keepT = np.ascontiguousarray(
            (m[b, 0] == 0).T.astype(bf16))                     # [t,s] bf16
        in_maps.append({"qT": qT, "kT": kT, "v": vv, "keepT": keepT})
    return in_maps


def _run(nc, in_maps):
    from concourse.bass_utils import run_bass_kernel_spmd
    return run_bass_kernel_spmd(nc, in_maps, list(range(NCORES)))


def kernel(query_layer, key_layer, value_layer, attention_mask):
    in_maps = _make_in_maps(query_layer, key_layer, value_layer, attention_mask)
    res = _run(_get_nc(1), in_maps)
    out = np.empty((SQ, B, NH, HN), dtype=np.float32)
    for c in range(NCORES):
        b = c // 4
        h0 = 4 * (c % 4)
        ctxT = np.asarray(res.results[c]["ctxT"], dtype=np.float32)   # [4,hn,sq]
        out[:, b, h0:h0 + HPC, :] = ctxT.transpose(2, 0, 1)
    return out.reshape(SQ, B, NH * HN)



# revision 2
# speedup vs baseline: 1.0009x; 1.0009x over previous
"""CoreAttention on 8 Trainium2 cores.

Sharding: 32 (batch, head) pairs -> 4 heads per core (cores 0-3: batch 0,
cores 4-7: batch 1). Per core, per head: scores^T = K Q^T in [t, s]
orientation with bf16 operands (fp32 PSUM accumulate), exp on ACT writing
bf16, mask applied as bf16 multiplies on DVE (2x perf mode, 4-chunk-wide
ops), column sums split: chunks 0..11 pair-tree-added on DVE into a
[128,2,SBLK] accumulator, chunks 12..15 plus the two accumulator rows
contracted on the PE with a ones vector. P@V as bf16 matmuls accumulating
in PSUM, normalization via reciprocal_approx_fast + gpsimd
partition_broadcast + DVE multiply.

Software-pipelined: scores/exp/mask for block i+1 are issued before the
PV/sums/normalize tail of block i so the PE never stalls on the DVE chain.
Input DMAs are issued in need-order as small pieces (32-131KB) so the
first score matmul's operands land within ~4us instead of waiting on a
half-MB transfer crawling on a single ~19GB/s DMA ring.
Host side only slices/transposes/casts inputs (layout prep).
"""
import sys, math
import numpy as np

sys.path.insert(0, "/opt/trn_rl_repo")

SQ, B, NH, HN = 2048, 2, 16, 128
NCORES = 8
HPC = 4                      # heads per core
TC = SQ // 128               # 16 t-chunks
SBLK = 512                   # s-block width
NSB = SQ // SBLK             # 4 s-blocks
SCALE = 1.0 / math.sqrt(128.0)   # COEFF / NORM_FACTOR = 1/sqrt(hn)
KDVE = 12                        # t-chunks whose column-sums go via DVE adds
DEPTH = 2                        # software pipeline depth (F blocks ahead of B)

_CACHE = {}


def _build(repeat=1):
    import concourse.bacc as bacc
    import concourse.tile as tile
    from concourse import mybir

    F32, BF16 = mybir.dt.float32, mybir.dt.bfloat16
    EXP = mybir.ActivationFunctionType.Exp

    nc = bacc.Bacc(None, target_bir_lowering=False)
    qT_d = nc.dram_tensor("qT", [HPC, HN, SQ], BF16, kind="ExternalInput")
    kT_d = nc.dram_tensor("kT", [HPC, HN, SQ], BF16, kind="ExternalInput")
    v_d = nc.dram_tensor("v", [HPC, SQ, HN], BF16, kind="ExternalInput")
    keep_d = nc.dram_tensor("keepT", [SQ, SQ], BF16, kind="ExternalInput")
    ctxT_d = nc.dram_tensor("ctxT", [HPC, HN, SQ], BF16, kind="ExternalOutput")

    with tile.TileContext(nc) as tc:
        with (
            tc.tile_pool(name="sbkeep", bufs=1) as sbkeep,
            tc.tile_pool(name="const", bufs=1) as const,
            tc.tile_pool(name="sbqkv", bufs=2) as sbqkv,
            tc.tile_pool(name="sbpt", bufs=DEPTH + 1) as sbpt,
            tc.tile_pool(name="sbacc", bufs=DEPTH + 1) as sbacc,
            tc.tile_pool(name="sbtmp", bufs=2) as sbtmp,
            tc.tile_pool(name="sbe", bufs=3) as sbe,
            tc.tile_pool(name="sbmisc", bufs=2) as sbmisc,
            tc.tile_pool(name="pst", bufs=2, space="PSUM") as pst,
            tc.tile_pool(name="psc", bufs=2, space="PSUM") as psc,
            tc.tile_pool(name="pss", bufs=1, space="PSUM") as pss,
            tc.tile_pool(name="pwm", bufs=1, space="PSUM") as pwm,
        ):
            keep_t = sbkeep.tile([128, TC, SQ], BF16, tag="keep")
            keep_r = keep_d.rearrange("(c p) s -> p c s", p=128)

            ones_b = const.tile([128, 1], BF16, tag="ob")
            nc.vector.memset(ones_b[:], 1.0)
            warm_src = const.tile([128, SBLK], BF16, tag="warm")
            nc.vector.memset(warm_src[:], 0.0)
            warm_e = const.tile([128, 16], BF16, tag="warme")

            def emit_front(h, sb, qT_t, kT_t):
                """scores -> exp -> mask for (h, sb); returns (pt, acc).

                Chunks 0..KDVE-1 are pair-tree-reduced on the DVE into
                acc[128, 2, SBLK] (wide FD=1024 bf16 adds); the PE later
                contracts only chunks KDVE..15 plus the two acc rows.
                """
                s0 = sb * SBLK
                pt = sbpt.tile([128, TC, SBLK], BF16, tag="pt")
                acc = sbacc.tile([128, 2, SBLK], BF16, tag="acc")
                tmp = sbtmp.tile([128, 2, SBLK], BF16, tag="tmp")
                for q in range(4):
                    e16 = sbe.tile([128, 4, SBLK], BF16, tag="e")
                    for half in range(2):
                        st = pst.tile([128, 2, SBLK], F32, tag="st")
                        for j in range(2):
                            ti = 4 * q + 2 * half + j
                            nc.tensor.matmul(
                                st[:, j, :],
                                kT_t[:, 128 * ti:128 * (ti + 1)],
                                qT_t[:, s0:s0 + SBLK],
                                start=True, stop=True)
                        nc.scalar.activation(
                            e16[:, 2 * half:2 * half + 2, :], st[:], EXP,
                            scale=SCALE)
                    nc.vector.tensor_mul(
                        pt[:, 4 * q:4 * q + 4, :], e16[:],
                        keep_t[:, 4 * q:4 * q + 4, s0:s0 + SBLK])
                    if q == 0:
                        nc.vector.tensor_add(acc[:], pt[:, 0:2, :],
                                             pt[:, 2:4, :])
                    elif q == 1:
                        nc.vector.tensor_add(tmp[:], pt[:, 4:6, :],
                                             pt[:, 6:8, :])
                    elif q == 2:
                        nc.vector.tensor_add(acc[:], acc[:], tmp[:])
                        nc.vector.tensor_add(tmp[:], pt[:, 8:10, :],
                                             pt[:, 10:12, :])
                    else:
                        nc.vector.tensor_add(acc[:], acc[:], tmp[:])
                return pt, acc

            def emit_back(h, sb, pt, acc, v_t, nsplit=1):
                """sums -> PV -> normalize -> store for (h, sb)."""
                s0 = sb * SBLK
                sums_p = pss.tile([1, SBLK], F32, tag="sums")
                for ti in range(KDVE, TC):
                    nc.tensor.matmul(sums_p[:], ones_b[:], pt[:, ti, :],
                                     start=(ti == KDVE), stop=False)
                nc.tensor.matmul(sums_p[:], ones_b[:], acc[:, 0, :],
                                 start=False, stop=False)
                nc.tensor.matmul(sums_p[:], ones_b[:], acc[:, 1, :],
                                 start=False, stop=True)
                ctx_p = psc.tile([128, SBLK], F32, tag="ctx")
                for ti in range(TC):
                    nc.tensor.matmul(ctx_p[:], v_t[:, ti, :], pt[:, ti, :],
                                     start=(ti == 0), stop=(ti == TC - 1))
                w = SBLK // nsplit
                for o in range(0, SBLK, w):
                    recip = sbmisc.tile([1, w], F32, tag="recip")
                    nc.vector.reciprocal_approx_fast(recip[:],
                                                     sums_p[:, o:o + w])
                    rep_s = sbmisc.tile([128, w], F32, tag="reps")
                    nc.gpsimd.partition_broadcast(rep_s[:], recip[:])
                    ctx_s = sbmisc.tile([128, w], BF16, tag="ctxs")
                    nc.vector.tensor_mul(ctx_s[:], ctx_p[:, o:o + w], rep_s[:])
                    nc.sync.dma_start(out=ctxT_d[h, :, s0 + o:s0 + o + w],
                                      in_=ctx_s[:])

            def body(_iv=None):
                # warm the PE clock (HAM) and the ACT exp table with dummy
                # ops that only depend on the memset, while the first DMAs
                # land
                warm_p = pwm.tile([1, SBLK], F32, tag="warmp")
                for _ in range(6):
                    nc.tensor.matmul(warm_p[:], ones_b[:], warm_src[:],
                                     start=True, stop=True)
                nc.scalar.activation(warm_e[:], warm_src[:, 0:16], EXP,
                                     scale=SCALE)

                pendings = []   # [(h, sb, pt, acc, v_t), ...]
                qkv = {}
                for h in range(HPC):
                    qT_t = sbqkv.tile([128, SQ], BF16, tag="qT")
                    kT_t = sbqkv.tile([128, SQ], BF16, tag="kT")
                    v_t = sbqkv.tile([128, TC, HN], BF16, tag="v")
                    if h == 0:
                        # need-ordered fine-grained loads: one DMA ring
                        # moves only ~19GB/s, so the first matmul's
                        # operands go out as 32KB pieces that land in
                        # ~2us, each on its own ring.
                        nc.sync.dma_start(out=kT_t[:, 0:128],
                                          in_=kT_d[h][:, 0:128])
                        for c in range(4):
                            nc.sync.dma_start(
                                out=qT_t[:, 128 * c:128 * (c + 1)],
                                in_=qT_d[h][:, 128 * c:128 * (c + 1)])
                        for c in range(1, 16):
                            nc.sync.dma_start(
                                out=kT_t[:, 128 * c:128 * (c + 1)],
                                in_=kT_d[h][:, 128 * c:128 * (c + 1)])
                        # mask columns for s-block 0 (block (0,0) masks
                        # need them first), v, then the later s-blocks'
                        # qT and mask columns in consumption order
                        for g in range(0, TC, 4):
                            nc.sync.dma_start(
                                out=keep_t[:, g:g + 4, 0:SBLK],
                                in_=keep_r[:, g:g + 4, 0:SBLK])
                        for half in range(2):
                            nc.sync.dma_start(
                                out=v_t[:, 8 * half:8 * (half + 1), :],
                                in_=v_d[h].rearrange(
                                    "(c p) d -> p c d",
                                    p=128)[:, 8 * half:8 * (half + 1), :])
                        for c in range(1, NSB):
                            nc.sync.dma_start(
                                out=qT_t[:, SBLK * c:SBLK * (c + 1)],
                                in_=qT_d[h][:, SBLK * c:SBLK * (c + 1)])
                        for sb in range(1, NSB):
                            s0 = sb * SBLK
                            for g in range(0, TC, 8):
                                nc.sync.dma_start(
                                    out=keep_t[:, g:g + 8, s0:s0 + SBLK],
                                    in_=keep_r[:, g:g + 8, s0:s0 + SBLK])
                    else:
                        for half in range(2):
                            cols = slice(SQ // 2 * half, SQ // 2 * (half + 1))
                            nc.sync.dma_start(out=qT_t[:, cols],
                                              in_=qT_d[h][:, cols])
                            nc.sync.dma_start(out=kT_t[:, cols],
                                              in_=kT_d[h][:, cols])
                            nc.sync.dma_start(
                                out=v_t[:, 8 * half:8 * (half + 1), :],
                                in_=v_d[h].rearrange(
                                    "(c p) d -> p c d",
                                    p=128)[:, 8 * half:8 * (half + 1), :])
                    qkv[h] = (qT_t, kT_t, v_t)
                    for sb in range(NSB):
                        pt, acc = emit_front(h, sb, qT_t, kT_t)
                        pendings.append((h, sb, pt, acc, v_t))
                        if len(pendings) > DEPTH:
                            emit_back(*pendings.pop(0))
                for i, p in enumerate(pendings):
                    emit_back(*p, nsplit=2 * (i + 1))

            if repeat == 1:
                body()
            else:
                with tc.For_i(0, repeat, 1):
                    body()
    nc.compile()
    return nc


def _get_nc(repeat=1):
    if repeat not in _CACHE:
        _CACHE[repeat] = _build(repeat)
    return _CACHE[repeat]


def _make_in_maps(query_layer, key_layer, value_layer, attention_mask):
    import ml_dtypes
    bf16 = ml_dtypes.bfloat16
    q = np.asarray(query_layer, dtype=np.float32)
    k = np.asarray(key_layer, dtype=np.float32)
    v = np.asarray(value_layer, dtype=np.float32)
    m = np.asarray(attention_mask)
    in_maps = []
    for c in range(NCORES):
        b = c // 4
        h0 = 4 * (c % 4)
        hs = slice(h0, h0 + HPC)
        qT = np.ascontiguousarray(
            q[:, b, hs, :].transpose(1, 2, 0)).astype(bf16)    # [4,hn,sq]
        kT = np.ascontiguousarray(
            k[:, b, hs, :].transpose(1, 2, 0)).astype(bf16)
        vv = np.ascontiguousarray(
            v[:, b, hs, :].transpose(1, 0, 2)).astype(bf16)    # [4,sq,hn]
        keepT = np.ascontiguousarray(
            (m[b, 0] == 0).T.astype(bf16))                     # [t,s] bf16
        in_maps.append({"qT": qT, "kT": kT, "v": vv, "keepT": keepT})
    return in_maps


def _run(nc, in_maps):
    from concourse.bass_utils import run_bass_kernel_spmd
    return run_bass_kernel_spmd(nc, in_maps, list(range(NCORES)))


def kernel(query_layer, key_layer, value_layer, attention_mask):
    in_maps = _make_in_maps(query_layer, key_layer, value_layer, attention_mask)
    res = _run(_get_nc(1), in_maps)
    out = np.empty((SQ, B, NH, HN), dtype=np.float32)
    for c in range(NCORES):
        b = c // 4
        h0 = 4 * (c % 4)
        ctxT = np.asarray(res.results[c]["ctxT"], dtype=np.float32)   # [4,hn,sq]
        out[:, b, h0:h0 + HPC, :] = ctxT.transpose(2, 0, 1)
    return out.reshape(SQ, B, NH * HN)
